# revision 25
# baseline (speedup 1.0000x reference)
"""GreedySampler kernel for 8 Trainium2 NeuronCores.

fp8 screen on device + exact host rescore of near-max candidates
(argmax(softmax(log(...))) = argmax(logits); fp8 logit error <=0.43
unscaled vs DELTA=2.0, so quantization only shortlists candidates).

v3. HW model (measured): a DoubleRow fp8 MATMUL streams 1 moving
column/cycle (206c for N=200 incl. pipeline gap; the PE fp8 peak is
128x256 MACs/cycle = 157TF/s), and LDWEIGHTS overlaps the previous
MATMUL almost fully. So the W-stationary schedule (one [K=256,M<=128]
W tile per pair, all 200 jobs moving) is at the PE MAC roofline:
ceil(6288/128)=50 tiles x 16 kk = 800 pairs ~= 69us. That exceeds the
~62us W DMA stream (~420GB/s on the sync HWDGE ring), so the kernel
is PE-bound: every PE stall and every half-clock cycle is wall time.

Per core (SPMD, vocab-sharded, groups 384+640x9+144 = 6288 cols):
  * Host packs the W shard into SBUF consumption order as one
    [P, bytes] partition-major tensor (multi-KB DMA descriptors; the
    16 shared DMA engines cost ~50ns/packet + ~30GB/s line rate).
  * All W on the sync HWDGE ring, kk-sliced fine at the stream head
    (the v1 baseline's 3.5us PE stall at t=13.5us waiting for group0
    kk8-15 also reset the HAM clock ramp to half speed until 22.7us;
    fine head chunks remove both penalties).
  * hst kk0-3 rides the sync ring first (lands ~9us, before the
    first real pair); the rest rides the gpsimd SWDGE ring whose
    completion sems cannot stall the W ring's 8-lane round-robin.
  * Short warmup (16 dummy pairs) starts the HAM clock ramp
    (0.65->2.4GHz) during the hst/W landing window.
  * kk-outer accumulation, one PSUM bank per 128-col tile, 8-bank
    rotation; mid-stream evictions on DVE only, outs on gpsimd SWDGE;
    the narrow 144-col last group splits DVE/ACT and ships on the
    then-idle scalar ring, cutting the post-PE tail to ~3us.

Walrus notes: instructions carrying >1 sync wait are rejected by this
build, so excess waits are split onto preceding nops; DoubleRow lhsT
slice widths must be 16B-aligned (128/16-wide subs only).
"""

import numpy as np
import ml_dtypes

import concourse.bass as bass
import concourse.mybir as mybir
import concourse.tile as tile
from concourse.vector_clock import ScopedClock
from concourse.bass_utils import run_bass_kernel_spmd

P = 128
N_CORES = 8
D = 4096
KK = D // 256  # 16 DoubleRow K-chunks of 256
W_SCALE = 32.0
DELTA = 2.0 * W_SCALE  # candidate margin in scaled-logit units

J = 200
VGS = [384] + [640] * 9 + [144]  # vocab-group widths per core
VS_EFF = sum(VGS)         # 6288
V_PAD = VS_EFF * N_CORES  # 50304 >= 50257

# kk-slice DMA cuts per group index: fine at the stream head, halves
# mid-stream (a PE catch-up at a group boundary then waits <=1.3MB,
# not a whole 2.6MB group)
W_CUTS = {0: [0, 4, 10, 16]}
W_CUTS_DEFAULT = [0, 8, 16]
N_WARMUP = 24

FP8 = mybir.dt.float8e4
F32 = mybir.dt.float32

_drain_patched = False


def _patch_tile_drain():
    """Split the tail Drain's sync waits (>1 rejected by this walrus)."""
    global _drain_patched
    if _drain_patched:
        return

    def _drain_and_barrier(self, tick_clock, wait_clock):
        nc = self.nc
        drain_inst = nc.sync.drain()
        wait_clock.add_sem_waits(
            drain_inst.ins, ScopedClock({None: tick_clock.global_clock})
        )
        si = drain_inst.ins.sync_info
        if si is not None and si.on_wait and len(si.on_wait) > 1:
            extra = list(si.on_wait[1:])
            del si.on_wait[1:]
            name2sem = {
                getattr(s, "name", None): s
                for s in self.sems.allocated().values()
            }
            for w in extra:
                nc.sync.wait_ge(name2sem[w.ant_name], w.wait_value)
        nc.all_engine_barrier()
        popped = nc._tile_sem_poison_stack.pop()
        assert popped is self._sem_poison
        nc.clear_and_free_semaphores(list(self.sems.allocated().values()))
        nc.all_engine_barrier()

    tile.TileContext._drain_and_barrier = _drain_and_barrier
    _drain_patched = True


def _split_excess_waits(nc, limit=1):
    """Move all but `limit` sync waits of every instruction onto nops
    inserted immediately before it on the same engine queue."""
    fn = nc.m.functions[0]
    for bb in fn.blocks:
        if not any(
            getattr(i, "sync_info", None) is not None
            and i.sync_info.on_wait
            and len(i.sync_info.on_wait) > limit
            for i in bb.instructions
        ):
            continue
        cur = nc.cur_bb.bb if hasattr(nc.cur_bb, "bb") else nc.cur_bb
        new_insts = []
        for inst in bb.instructions:
            si = getattr(inst, "sync_info", None)
            if si is not None and si.on_wait and len(si.on_wait) > limit:
                extra = list(si.on_wait[:-limit])
                del si.on_wait[: len(si.on_wait) - limit]
                for w in extra:
                    nop = nc.engines[inst.engine].nop(nofuse=True).ins
                    popped = cur.instructions.pop()  # nop() self-appended
                    assert popped is nop
                    nop.sync_info = mybir.SyncInfo(on_wait=[w], on_update=[])
                    new_insts.append(nop)
            new_insts.append(inst)
        bb.instructions[:] = new_insts


def _sub_widths(w):
    subs = [P] * (w // P)
    if w % P:
        subs.append(w % P)
    return subs


NSUBS = [len(_sub_widths(w)) for w in VGS]
OUT_TOT = sum(NSUBS) * J


def build_nc():
    _patch_tile_drain()

    nc = bass.Bass()
    hst = nc.dram_tensor("hst", [P, KK, 2, J], FP8, kind="ExternalInput")
    wt = nc.dram_tensor("wt", [P, KK * 2 * VS_EFF], FP8, kind="ExternalInput")
    lg = nc.dram_tensor("lg", [P, OUT_TOT], FP8, kind="ExternalOutput")

    with tile.TileContext(nc) as tc:
        with (
            tc.tile_pool(name="hs", bufs=1) as hs_pool,
            tc.tile_pool(name="w", bufs=8) as w_pool,
            tc.tile_pool(name="out", bufs=4) as out_pool,
            tc.tile_pool(name="ps", bufs=8, space=bass.MemorySpace.PSUM) as ps_pool,
        ):
            # hst rides the sync HWDGE ring (kk0-9, 3 pieces, lands by
            # ~10.5us) while the scalar ring alone carries the early W
            # (a single HWDGE queue saturates all 16 DMA engines, so
            # nothing is lost while the queues specialize); kk10-15 on
            # gpsimd SWDGE (~3us spin-up, needed only at ~13us)
            hst_sb = hs_pool.tile([P, KK, 2, J], FP8)
            wu_w = out_pool.tile([P, 2, P], FP8, name="wu_w")
            wu_h = out_pool.tile([P, 2, J], FP8, name="wu_h")
            wu_d = out_pool.tile([1, 1], FP8, name="wu_d")
            nc.vector.memset(wu_w[:], 0.0)
            nc.vector.memset(wu_h[:], 0.0)
            nc.gpsimd.dma_start(hst_sb[:, 10:KK], hst[:, 10:KK])

            # W stream alternates chunks between the sync and scalar
            # HWDGE queues: two queues keep more packets outstanding on
            # the 16 shared DMA engines, hardening against the
            # 335-425GB/s run-to-run HBM weather that otherwise stalls
            # the PE mid-stream (the PE needs >=375GB/s sustained).
            # The first chunks are planned explicitly so the scalar
            # queue's W g0 head chunks issue with nothing ahead of them.
            w_sbs = [None] * len(VGS)
            woffs = []
            woff = 0
            for wv in VGS:
                woffs.append(woff)
                woff += KK * 2 * wv

            def w_chunk(ring, vg, a, e):
                wv = VGS[vg]
                if w_sbs[vg] is None:
                    w_sbs[vg] = w_pool.tile([P, KK, 2, wv], FP8, name="w_sb")
                src = wt[:, woffs[vg] + a * 2 * wv: woffs[vg] + e * 2 * wv]
                ring.dma_start(
                    w_sbs[vg][:, a:e],
                    src.rearrange("p (k t w) -> p k t w", k=e - a, t=2),
                )

            # dummy activation FIRST on scalar: the lazy ACT_TABLE_LOAD
            # (~1.3us) runs at ~7.5us before the queue's W issues, not
            # right before the last group's scalar.copy on the tail
            nc.scalar.copy(wu_d[:], wu_w[:1, 0, :1])
            # sync carries g0+hst (consumed first), scalar g1/g2 heads
            w_chunk(nc.sync, 0, 0, 4)
            nc.sync.dma_start(hst_sb[:, 0:2], hst[:, 0:2])
            nc.sync.dma_start(hst_sb[:, 2:4], hst[:, 2:4])
            w_chunk(nc.scalar, 1, 0, 8)
            w_chunk(nc.sync, 0, 4, 10)
            nc.sync.dma_start(hst_sb[:, 4:10], hst[:, 4:10])
            w_chunk(nc.scalar, 2, 0, 8)
            w_chunk(nc.sync, 0, 10, 16)
            w_chunk(nc.scalar, 1, 8, 16)
            w_chunk(nc.sync, 2, 8, 16)
            nch = 0
            for vg in range(3, len(VGS)):
                cuts = W_CUTS.get(vg, W_CUTS_DEFAULT)
                for a, e in zip(cuts[:-1], cuts[1:]):
                    # the last group rides sync: the scalar queue turns
                    # into the out ring at the stream end
                    if vg == len(VGS) - 1:
                        ring = nc.sync
                    else:
                        ring = nc.scalar if nch % 2 == 0 else nc.sync
                    w_chunk(ring, vg, a, e)
                    nch += 1

            ooff = 0
            for vg, wv in enumerate(VGS):
                subs = _sub_widths(wv)
                w_sb = w_sbs[vg]
                last = vg == len(VGS) - 1
                pss = [ps_pool.tile([P, 512], F32, name="ps") for _ in subs]
                if vg == 0:
                    # complete (start+stop) dummy groups; the bank is
                    # free again before the real kk=0 accumulation
                    for _ in range(N_WARMUP):
                        nc.tensor.matmul(
                            pss[0][:, :J], wu_w[:], wu_h[:],
                            start=True, stop=True,
                            perf_mode=mybir.MatmulPerfMode.DoubleRow,
                        )
                for kk in range(KK):
                    soff = 0
                    for s, sw in enumerate(subs):
                        nc.tensor.matmul(
                            pss[s][:sw, :J],
                            w_sb[:, kk, :, soff:soff + sw],
                            hst_sb[:, kk, :, :],
                            start=(kk == 0),
                            stop=(kk == KK - 1),
                            perf_mode=mybir.MatmulPerfMode.DoubleRow,
                        )
                        soff += sw
                # evictions: DVE only mid-stream (ACT/scalar queue kept
                # clear); the post-stream last group splits DVE/ACT so
                # the tail drains in parallel
                ot = out_pool.tile([P, len(subs), J], FP8, name="ot")
                if last:
                    # memset backfills the ragged sub's unused rows so
                    # the tail ships as ONE dma (no serialized issues)
                    nc.vector.memset(ot[:], 0.0)
                for s, sw in enumerate(subs):
                    if last and s % 2 == 1:
                        nc.scalar.copy(ot[:sw, s, :], pss[s][:sw, :J])
                    else:
                        nc.vector.tensor_copy(ot[:sw, s, :], pss[s][:sw, :J])
                # outs: gpsimd SWDGE mid-stream (its completion sems
                # cannot stall the W ring); the final two groups use the
                # then-idle scalar/sync HWDGE rings
                if last:
                    nc.scalar.dma_start(
                        lg[:, ooff:ooff + len(subs) * J],
                        ot[:].rearrange("p s j -> p (s j)"),
                    )
                    ooff += len(subs) * J
                    continue
                ring = nc.sync if vg == len(VGS) - 2 else nc.gpsimd
                nfull = sum(1 for sw in subs if sw == P)
                if nfull == len(subs):
                    ring.dma_start(
                        lg[:, ooff:ooff + nfull * J],
                        ot[:].rearrange("p s j -> p (s j)"),
                    )
                else:
                    ring.dma_start(
                        lg[:, ooff:ooff + nfull * J],
                        ot[:, :nfull, :].rearrange("p s j -> p (s j)"),
                    )
                    sw = subs[-1]
                    ring.dma_start(
                        lg[:sw, ooff + nfull * J:ooff + (nfull + 1) * J],
                        ot[:sw, nfull, :],
                    )
                ooff += len(subs) * J

    _split_excess_waits(nc, limit=1)
    return nc


def _pack_w(shard):
    """shard [D, VS_EFF] fp8 -> [P, bytes] partition-major, vg-blocked,
    contiguous in DMA consumption order."""
    blocks = []
    off = 0
    for wv in VGS:
        a = shard[:, off:off + wv].reshape(KK, 2, P, wv)
        blocks.append(np.ascontiguousarray(
            a.transpose(2, 0, 1, 3)).reshape(P, -1))
        off += wv
    return np.concatenate(blocks, axis=1)


def _decode_logits(lgbuf):
    """[P, OUT_TOT] fp8 -> [J, VS_EFF] f32."""
    res = np.empty((J, VS_EFF), np.float32)
    o = 0
    c = 0
    arr = lgbuf.astype(np.float32)
    for vg, wv in enumerate(VGS):
        for sw in _sub_widths(wv):
            res[:, c:c + sw] = arr[:sw, o:o + J].T
            o += J
            c += sw
    return res


def _job_indices(fill_tokens_num, num_generation_jobs):
    fill = np.asarray(fill_tokens_num, dtype=np.int64)
    fill_last = np.cumsum(fill) - 1
    total_fill = int(fill.sum())
    gen = total_fill + np.arange(int(num_generation_jobs), dtype=np.int64)
    return np.concatenate([fill_last, gen])


def kernel(hidden_states, embd_weight, fill_tokens_num, num_generation_jobs):
    hs = np.asarray(hidden_states, dtype=np.float32)
    W = np.asarray(embd_weight, dtype=np.float32)
    V, Dd = W.shape

    idx = _job_indices(fill_tokens_num, num_generation_jobs)
    assert idx.size == J

    hs_sel = hs[idx]
    hst_host = np.ascontiguousarray(
        hs_sel.T.reshape(KK, 2, P, J).transpose(2, 0, 1, 3)
    ).astype(ml_dtypes.float8_e4m3)

    Wq = (W * W_SCALE).astype(ml_dtypes.float8_e4m3)
    WT_pad = np.zeros((Dd, V_PAD), dtype=ml_dtypes.float8_e4m3)
    WT_pad[:, :V] = Wq.T
    shards = [
        _pack_w(WT_pad[:, i * VS_EFF:(i + 1) * VS_EFF]) for i in range(N_CORES)
    ]

    nc = build_nc()
    kernel.last_nc = nc
    kernel.last_in_maps = [
        {"hst": hst_host, "wt": shards[i]} for i in range(N_CORES)
    ]
    res = run_bass_kernel_spmd(
        nc, kernel.last_in_maps, core_ids=list(range(N_CORES))
    )
    kernel.last_results = res

    logits = np.concatenate(
        [_decode_logits(res.results[i]["lg"]) for i in range(N_CORES)],
        axis=1,
    )[:, :V]
    logits = np.where(np.isnan(logits), np.inf, logits)

    m = logits.max(axis=1, keepdims=True)
    rows, cols = np.nonzero(logits >= m - DELTA)
    exact = np.einsum(
        "ij,ij->i", hs_sel[rows].astype(np.float64), W[cols].astype(np.float64)
    )
    ids = np.zeros(J, dtype=np.int64)
    best = np.full(J, -np.inf)
    for r, c, s in zip(rows, cols, exact):
        if s > best[r]:
            best[r] = s
            ids[r] = c
    return ids.astype(np.int32)


# revision 27
# speedup vs baseline: 1.0705x; 1.0705x over previous
"""GreedySampler kernel for 8 Trainium2 NeuronCores.

fp8 screen on device + exact host rescore of near-max candidates
(argmax(softmax(log(...))) = argmax(logits); fp8 logit error <=0.43
unscaled vs DELTA=2.0, so quantization only shortlists candidates).

v3. HW model (measured): a DoubleRow fp8 MATMUL streams 1 moving
column/cycle (206c for N=200 incl. pipeline gap; the PE fp8 peak is
128x256 MACs/cycle = 157TF/s), and LDWEIGHTS overlaps the previous
MATMUL almost fully. So the W-stationary schedule (one [K=256,M<=128]
W tile per pair, all 200 jobs moving) is at the PE MAC roofline:
ceil(6288/128)=50 tiles x 16 kk = 800 pairs ~= 69us. That exceeds the
~62us W DMA stream (~420GB/s on the sync HWDGE ring), so the kernel
is PE-bound: every PE stall and every half-clock cycle is wall time.

Per core (SPMD, vocab-sharded, groups 384+640x9+144 = 6288 cols):
  * Host packs the W shard into SBUF consumption order as one
    [P, bytes] partition-major tensor (multi-KB DMA descriptors; the
    16 shared DMA engines cost ~50ns/packet + ~30GB/s line rate).
  * All W on the sync HWDGE ring, kk-sliced fine at the stream head
    (the v1 baseline's 3.5us PE stall at t=13.5us waiting for group0
    kk8-15 also reset the HAM clock ramp to half speed until 22.7us;
    fine head chunks remove both penalties).
  * hst kk0-3 rides the sync ring first (lands ~9us, before the
    first real pair); the rest rides the gpsimd SWDGE ring whose
    completion sems cannot stall the W ring's 8-lane round-robin.
  * Short warmup (16 dummy pairs) starts the HAM clock ramp
    (0.65->2.4GHz) during the hst/W landing window.
  * kk-outer accumulation, one PSUM bank per 128-col tile, 8-bank
    rotation; mid-stream evictions on DVE only, outs on gpsimd SWDGE;
    the narrow 144-col last group splits DVE/ACT and ships on the
    then-idle scalar ring, cutting the post-PE tail to ~3us.

Walrus notes: instructions carrying >1 sync wait are rejected by this
build, so excess waits are split onto preceding nops; DoubleRow lhsT
slice widths must be 16B-aligned (128/16-wide subs only).
"""

import numpy as np
import ml_dtypes

import concourse.bass as bass
import concourse.mybir as mybir
import concourse.tile as tile
from concourse.vector_clock import ScopedClock
from concourse.bass_utils import run_bass_kernel_spmd

P = 128
N_CORES = 8
D = 4096
KK = D // 256  # 16 DoubleRow K-chunks of 256
W_SCALE = 32.0
DELTA = 2.0 * W_SCALE  # candidate margin in scaled-logit units

J = 200
VGS = [384] + [640] * 9 + [144]  # vocab-group widths per core
VS_EFF = sum(VGS)         # 6288
V_PAD = VS_EFF * N_CORES  # 50304 >= 50257

# kk-slice DMA cuts per group index: fine at the stream head, halves
# mid-stream (a PE catch-up at a group boundary then waits <=1.3MB,
# not a whole 2.6MB group)
W_CUTS = {0: [0, 2, 4, 8, 16], 1: [0, 4, 8, 16]}
W_CUTS_DEFAULT = [0, 8, 16]
N_WARMUP = 24

FP8 = mybir.dt.float8e4
F32 = mybir.dt.float32

_drain_patched = False


def _patch_tile_drain():
    """Split the tail Drain's sync waits (>1 rejected by this walrus)."""
    global _drain_patched
    if _drain_patched:
        return

    def _drain_and_barrier(self, tick_clock, wait_clock):
        nc = self.nc
        drain_inst = nc.sync.drain()
        wait_clock.add_sem_waits(
            drain_inst.ins, ScopedClock({None: tick_clock.global_clock})
        )
        si = drain_inst.ins.sync_info
        if si is not None and si.on_wait and len(si.on_wait) > 1:
            extra = list(si.on_wait[1:])
            del si.on_wait[1:]
            name2sem = {
                getattr(s, "name", None): s
                for s in self.sems.allocated().values()
            }
            for w in extra:
                nc.sync.wait_ge(name2sem[w.ant_name], w.wait_value)
        nc.all_engine_barrier()
        popped = nc._tile_sem_poison_stack.pop()
        assert popped is self._sem_poison
        nc.clear_and_free_semaphores(list(self.sems.allocated().values()))
        nc.all_engine_barrier()

    tile.TileContext._drain_and_barrier = _drain_and_barrier
    _drain_patched = True


def _split_excess_waits(nc, limit=1):
    """Move all but `limit` sync waits of every instruction onto nops
    inserted immediately before it on the same engine queue."""
    fn = nc.m.functions[0]
    for bb in fn.blocks:
        if not any(
            getattr(i, "sync_info", None) is not None
            and i.sync_info.on_wait
            and len(i.sync_info.on_wait) > limit
            for i in bb.instructions
        ):
            continue
        cur = nc.cur_bb.bb if hasattr(nc.cur_bb, "bb") else nc.cur_bb
        new_insts = []
        for inst in bb.instructions:
            si = getattr(inst, "sync_info", None)
            if si is not None and si.on_wait and len(si.on_wait) > limit:
                extra = list(si.on_wait[:-limit])
                del si.on_wait[: len(si.on_wait) - limit]
                for w in extra:
                    nop = nc.engines[inst.engine].nop(nofuse=True).ins
                    popped = cur.instructions.pop()  # nop() self-appended
                    assert popped is nop
                    nop.sync_info = mybir.SyncInfo(on_wait=[w], on_update=[])
                    new_insts.append(nop)
            new_insts.append(inst)
        bb.instructions[:] = new_insts


def _sub_widths(w):
    subs = [P] * (w // P)
    if w % P:
        subs.append(w % P)
    return subs


NSUBS = [len(_sub_widths(w)) for w in VGS]
OUT_TOT = sum(NSUBS) * J


def build_nc():
    _patch_tile_drain()

    nc = bass.Bass()
    hst = nc.dram_tensor("hst", [P, KK, 2, J], FP8, kind="ExternalInput")
    wt = nc.dram_tensor("wt", [P, KK * 2 * VS_EFF], FP8, kind="ExternalInput")
    lg = nc.dram_tensor("lg", [P, OUT_TOT], FP8, kind="ExternalOutput")

    with tile.TileContext(nc) as tc:
        with (
            tc.tile_pool(name="hs", bufs=1) as hs_pool,
            tc.tile_pool(name="w", bufs=8) as w_pool,
            tc.tile_pool(name="out", bufs=4) as out_pool,
            tc.tile_pool(name="ps", bufs=8, space=bass.MemorySpace.PSUM) as ps_pool,
        ):
            # hst rides the sync HWDGE ring (kk0-9, 3 pieces, lands by
            # ~10.5us) while the scalar ring alone carries the early W
            # (a single HWDGE queue saturates all 16 DMA engines, so
            # nothing is lost while the queues specialize); kk10-15 on
            # gpsimd SWDGE (~3us spin-up, needed only at ~13us)
            hst_sb = hs_pool.tile([P, KK, 2, J], FP8)
            wu_w = out_pool.tile([P, 2, P], FP8, name="wu_w")
            wu_h = out_pool.tile([P, 2, J], FP8, name="wu_h")
            wu_d = out_pool.tile([1, 1], FP8, name="wu_d")
            nc.vector.memset(wu_w[:], 0.0)
            nc.vector.memset(wu_h[:], 0.0)
            nc.gpsimd.dma_start(hst_sb[:, 10:KK], hst[:, 10:KK])

            # W stream alternates chunks between the sync and scalar
            # HWDGE queues: two queues keep more packets outstanding on
            # the 16 shared DMA engines, hardening against the
            # 335-425GB/s run-to-run HBM weather that otherwise stalls
            # the PE mid-stream (the PE needs >=375GB/s sustained).
            # The first chunks are planned explicitly so the scalar
            # queue's W g0 head chunks issue with nothing ahead of them.
            w_sbs = [None] * len(VGS)
            woffs = []
            woff = 0
            for wv in VGS:
                woffs.append(woff)
                woff += KK * 2 * wv

            def w_chunk(ring, vg, a, e):
                wv = VGS[vg]
                if w_sbs[vg] is None:
                    w_sbs[vg] = w_pool.tile([P, KK, 2, wv], FP8, name="w_sb")
                src = wt[:, woffs[vg] + a * 2 * wv: woffs[vg] + e * 2 * wv]
                ring.dma_start(
                    w_sbs[vg][:, a:e],
                    src.rearrange("p (k t w) -> p k t w", k=e - a, t=2),
                )

            # scalar carries g0's head chunks with nothing ahead of
            # them; sync carries hst (consumed in kk order)
            w_chunk(nc.scalar, 0, 0, 2)
            nc.sync.dma_start(hst_sb[:, 0:2], hst[:, 0:2])
            w_chunk(nc.scalar, 0, 2, 4)
            nc.sync.dma_start(hst_sb[:, 2:4], hst[:, 2:4])
            w_chunk(nc.scalar, 0, 4, 8)
            nc.sync.dma_start(hst_sb[:, 4:10], hst[:, 4:10])
            w_chunk(nc.scalar, 0, 8, 16)
            w_chunk(nc.sync, 1, 0, 4)
            w_chunk(nc.scalar, 1, 4, 8)
            w_chunk(nc.sync, 1, 8, 16)
            # dummy activation here: the lazy ACT_TABLE_LOAD (~1.5us)
            # runs at ~12us on the scalar queue between W issues, not
            # right before the last group's scalar.copy on the tail
            nc.scalar.copy(wu_d[:], wu_w[:1, 0, :1])
            nch = 0
            for vg in range(2, len(VGS)):
                cuts = W_CUTS.get(vg, W_CUTS_DEFAULT)
                for a, e in zip(cuts[:-1], cuts[1:]):
                    # the last group rides sync: the scalar queue turns
                    # into the out ring at the stream end
                    if vg == len(VGS) - 1:
                        ring = nc.sync
                    else:
                        ring = nc.sync if nch % 2 == 0 else nc.scalar
                    w_chunk(ring, vg, a, e)
                    nch += 1

            ooff = 0
            for vg, wv in enumerate(VGS):
                subs = _sub_widths(wv)
                w_sb = w_sbs[vg]
                last = vg == len(VGS) - 1
                pss = [ps_pool.tile([P, 512], F32, name="ps") for _ in subs]
                if vg == 0:
                    # complete (start+stop) dummy groups; the bank is
                    # free again before the real kk=0 accumulation
                    for _ in range(N_WARMUP):
                        nc.tensor.matmul(
                            pss[0][:, :J], wu_w[:], wu_h[:],
                            start=True, stop=True,
                            perf_mode=mybir.MatmulPerfMode.DoubleRow,
                        )
                for kk in range(KK):
                    soff = 0
                    for s, sw in enumerate(subs):
                        nc.tensor.matmul(
                            pss[s][:sw, :J],
                            w_sb[:, kk, :, soff:soff + sw],
                            hst_sb[:, kk, :, :],
                            start=(kk == 0),
                            stop=(kk == KK - 1),
                            perf_mode=mybir.MatmulPerfMode.DoubleRow,
                        )
                        soff += sw
                # evictions: DVE only mid-stream (ACT/scalar queue kept
                # clear); the post-stream last group splits DVE/ACT so
                # the tail drains in parallel
                ot = out_pool.tile([P, len(subs), J], FP8, name="ot")
                if last:
                    # memset backfills the ragged sub's unused rows so
                    # the tail ships as ONE dma (no serialized issues)
                    nc.vector.memset(ot[:], 0.0)
                for s, sw in enumerate(subs):
                    if last and s % 2 == 1:
                        nc.scalar.copy(ot[:sw, s, :], pss[s][:sw, :J])
                    else:
                        nc.vector.tensor_copy(ot[:sw, s, :], pss[s][:sw, :J])
                # outs: gpsimd SWDGE mid-stream (its completion sems
                # cannot stall the W ring); the final two groups use the
                # then-idle scalar/sync HWDGE rings
                if last:
                    nc.scalar.dma_start(
                        lg[:, ooff:ooff + len(subs) * J],
                        ot[:].rearrange("p s j -> p (s j)"),
                    )
                    ooff += len(subs) * J
                    continue
                ring = nc.sync if vg == len(VGS) - 2 else nc.gpsimd
                nfull = sum(1 for sw in subs if sw == P)
                if nfull == len(subs):
                    ring.dma_start(
                        lg[:, ooff:ooff + nfull * J],
                        ot[:].rearrange("p s j -> p (s j)"),
                    )
                else:
                    ring.dma_start(
                        lg[:, ooff:ooff + nfull * J],
                        ot[:, :nfull, :].rearrange("p s j -> p (s j)"),
                    )
                    sw = subs[-1]
                    ring.dma_start(
                        lg[:sw, ooff + nfull * J:ooff + (nfull + 1) * J],
                        ot[:sw, nfull, :],
                    )
                ooff += len(subs) * J

    _split_excess_waits(nc, limit=1)
    return nc


def _pack_w(shard):
    """shard [D, VS_EFF] fp8 -> [P, bytes] partition-major, vg-blocked,
    contiguous in DMA consumption order."""
    blocks = []
    off = 0
    for wv in VGS:
        a = shard[:, off:off + wv].reshape(KK, 2, P, wv)
        blocks.append(np.ascontiguousarray(
            a.transpose(2, 0, 1, 3)).reshape(P, -1))
        off += wv
    return np.concatenate(blocks, axis=1)


def _decode_logits(lgbuf):
    """[P, OUT_TOT] fp8 -> [J, VS_EFF] f32."""
    res = np.empty((J, VS_EFF), np.float32)
    o = 0
    c = 0
    arr = lgbuf.astype(np.float32)
    for vg, wv in enumerate(VGS):
        for sw in _sub_widths(wv):
            res[:, c:c + sw] = arr[:sw, o:o + J].T
            o += J
            c += sw
    return res


def _job_indices(fill_tokens_num, num_generation_jobs):
    fill = np.asarray(fill_tokens_num, dtype=np.int64)
    fill_last = np.cumsum(fill) - 1
    total_fill = int(fill.sum())
    gen = total_fill + np.arange(int(num_generation_jobs), dtype=np.int64)
    return np.concatenate([fill_last, gen])


def kernel(hidden_states, embd_weight, fill_tokens_num, num_generation_jobs):
    hs = np.asarray(hidden_states, dtype=np.float32)
    W = np.asarray(embd_weight, dtype=np.float32)
    V, Dd = W.shape

    idx = _job_indices(fill_tokens_num, num_generation_jobs)
    assert idx.size == J

    hs_sel = hs[idx]
    hst_host = np.ascontiguousarray(
        hs_sel.T.reshape(KK, 2, P, J).transpose(2, 0, 1, 3)
    ).astype(ml_dtypes.float8_e4m3)

    Wq = (W * W_SCALE).astype(ml_dtypes.float8_e4m3)
    WT_pad = np.zeros((Dd, V_PAD), dtype=ml_dtypes.float8_e4m3)
    WT_pad[:, :V] = Wq.T
    shards = [
        _pack_w(WT_pad[:, i * VS_EFF:(i + 1) * VS_EFF]) for i in range(N_CORES)
    ]

    nc = build_nc()
    kernel.last_nc = nc
    kernel.last_in_maps = [
        {"hst": hst_host, "wt": shards[i]} for i in range(N_CORES)
    ]
    res = run_bass_kernel_spmd(
        nc, kernel.last_in_maps, core_ids=list(range(N_CORES))
    )
    kernel.last_results = res

    logits = np.concatenate(
        [_decode_logits(res.results[i]["lg"]) for i in range(N_CORES)],
        axis=1,
    )[:, :V]
    logits = np.where(np.isnan(logits), np.inf, logits)

    m = logits.max(axis=1, keepdims=True)
    rows, cols = np.nonzero(logits >= m - DELTA)
    exact = np.einsum(
        "ij,ij->i", hs_sel[rows].astype(np.float64), W[cols].astype(np.float64)
    )
    ids = np.zeros(J, dtype=np.int64)
    best = np.full(J, -np.inf)
    for r, c, s in zip(rows, cols, exact):
        if s > best[r]:
            best[r] = s
            ids[r] = c
    return ids.astype(np.int32)
